# revision 4
# baseline (speedup 1.0000x reference)
"""Trainium2 Bass kernel for the NT-Xent / CLIP-style contrastive loss.

Reference computation (N=8192, D=512, fp32):
    zi_n, zj_n = row-normalize(z_i), row-normalize(z_j)
    sim = zi_n @ zj_n.T / TAU
    loss_e2t = mean_i( logsumexp_{j!=i}(sim[i,:]) - sim[i,i] )
    loss_t2e = mean_j( logsumexp_{i!=j}(sim[:,j]) - sim[j,j] )
    out = [ (loss_e2t+loss_t2e)/2, loss_e2t, loss_t2e ]

Sharding: rows of z_i are split across the 8 cores (1024 rows each); the
normalized z_j is replicated (the host plays the role of the all-gather).
Each core computes its [1024, 8192] tile of exp(sim); row sums feed
lse_row, 128-partial column sums feed lse_col, and the host finishes the
128-way + 8-core reduction plus the final log/mean epilogue.

The design goal is a never-stalling TensorE (the PE matmul stream is the
theoretical floor at ~55us/core). Each [128, 2048] column group per row
chunk is computed into TWO independent PSUM tiles so their consumers
release them separately:
  * gpA cols [0:1024]   -> ScalarE table exp (+fused accum_out row sums).
    Columns accumulate into colboth: [0:KCV] on VectorE, [KCV:1024] on
    GpSimd (split tuned so both engines run just under the PE period).
  * gpB cols [1024:2048] -> VectorE Schraudolph fast exp -- a single
    tensor_scalar computing uint8(round(x*A + B)) whose bit pattern IS
    the fp8e4m3 exp approximation. Two consecutive row chunks share one
    [P, 2, HC] tile so a single DMA ships 256KB (halves the ~650ns/DMA
    HWDGE dispatch cost on the sync ring). The host reduces these into
    both row and column sums in fp64.

DMA dispatch (~650ns per 128-partition DMA regardless of size) is split
across the two HWDGE rings: steady-state outbound rides sync; the scalar
ring takes the zj k=1 input chunks at the head and the final raw-et +
rowsums ships at the tail, so neither ring serializes the critical path.
The last slot (g=3, rc=7) skips the on-device column accumulate and ships
its et tile raw (host adds it), removing two engine ops + a dependent
ship from the drain tail.

Main matmul runs in fp8e4m3 with DoubleRow packing (2 contraction rows per
PE cell). Operands are scaled by 32 before the fp8 cast to stay clear of
denormals; the 1/32^2 is folded into the exp scale.
"""

import math
import os
import sys

for _p in ("/opt/trn_rl_repo", "/root/.axon_site/_ro/trn_rl_repo"):
    if os.path.isdir(_p) and _p not in sys.path:
        sys.path.insert(0, _p)

import numpy as np
import ml_dtypes

import concourse.bass as bass
import concourse.bacc as bacc
import concourse.mybir as mybir
import concourse.tile as tile
from concourse import bass_utils

TAU = 0.07
EPS = 1e-8

N = 8192            # batch
D = 512             # embed dim
NCORES = 8
NI = N // NCORES    # rows per core (1024)
P = 128             # partitions
RC = NI // P        # row chunks per core (8)
CCG = 2048          # columns per group (one iteration)
NCCG = N // CCG     # 4 groups
MMN = 512           # matmul moving size (one PSUM bank of fp32)
NS = RC * NCCG      # accum slots
HC = 1024           # columns per PSUM half-tile
KC = 1024           # colacc columns per group (device-accumulated)
KCV = 224           # colacc columns accumulated on VectorE
KCP = KC - KCV      # colacc columns accumulated on GpSimd

FP8_SCALE = 32.0
# exp argument = psum * ES (psum carries the 32^2 fp8 pre-scale)
ES = 1.0 / (TAU * FP8_SCALE * FP8_SCALE)

# Schraudolph uint8/fp8e4m3 fast exp: fp8_bits(exp(y)) ~= round(y*8/ln2 + B)
# (3-bit mantissa -> 8 steps per octave; bias 7 -> 56 at y=0)
SCHRAUDOLPH_A = 8.0 / math.log(2.0) * ES
SCHRAUDOLPH_B = 56.0 - 0.46  # C=0.46 zeroes the mean bias (numpy scan)

BF16 = mybir.dt.bfloat16
F32 = mybir.dt.float32
FP8 = mybir.dt.float8e4
U8 = mybir.dt.uint8
NP_FP8 = mybir.dt.np(FP8)

LAST_RESULTS = None  # BassKernelResults of the most recent run (for test.py)

_compiled = {}


def _build():
    """Build + compile the single-core SPMD Bass program."""
    nc = bacc.Bacc("TRN2", target_bir_lowering=False, debug=False)

    # zi: [half, p, k, s, c] with contraction row d = k*256 + s*128 + p and
    # column = half*512 + c; each half is one fully-contiguous 256KB DMA.
    # zj: [k, p, s, n] -- every dispatch slice has >=1KB descriptor runs.
    zi_t = nc.dram_tensor("zi_t", [2, P, 2, 2, NI // 2], FP8,
                          kind="ExternalInput")
    zjh_t = nc.dram_tensor("zjh_t", [2, P, 2, N], FP8, kind="ExternalInput")
    rows_d = nc.dram_tensor("rowsums", [P, NS], F32, kind="ExternalOutput")
    cols_d = nc.dram_tensor("colacc", [NCCG, P, KC], BF16,
                            kind="ExternalOutput")
    etf_d = nc.dram_tensor("etf", [NCCG, RC // 2, P, 2, HC], U8,
                           kind="ExternalOutput")
    etl_d = nc.dram_tensor("etl", [P, HC], BF16, kind="ExternalOutput")

    with tile.TileContext(nc) as tc:
        _body(nc, tc, zi_t.ap(), zjh_t.ap(), rows_d.ap(), cols_d.ap(),
              etf_d.ap(), etl_d.ap())

    nc.compile()
    return nc


def _body(nc, tc, zi_t, zjh_t, rows_d, cols_d, etf_d, etl_d):
    from contextlib import ExitStack

    perf_mode = mybir.MatmulPerfMode.DoubleRow

    with ExitStack() as ctx:
        zpool = ctx.enter_context(tc.tile_pool(name="z", bufs=1))
        epool = ctx.enter_context(tc.tile_pool(name="e", bufs=10))
        apool = ctx.enter_context(tc.tile_pool(name="acc", bufs=1))
        psa = ctx.enter_context(
            tc.tile_pool(name="psa", bufs=2, space=bass.MemorySpace.PSUM)
        )
        psb = ctx.enter_context(
            tc.tile_pool(name="psb", bufs=2, space=bass.MemorySpace.PSUM)
        )

        # ---- PE clock warmup ------------------------------------------
        # Short dummy DoubleRow matmuls keep the PE busy from the moment
        # the preamble ends until the first inputs land (~1.5us), so the
        # HAM clock gate (1.2 -> 2.4 GHz, ~4.8us of sustained activity)
        # opens as early as possible. The memset rides VectorE, which is
        # otherwise idle at the head (GpSimd has the colboth memset).
        wsrc = zpool.tile([P, 2, 256], FP8, tag="wsrc", name="wsrc")
        nc.vector.memset(wsrc[:], 0)
        wp = psa.tile([P, HC], F32, tag="GA", name="warm")
        for w in range(8):
            nc.tensor.matmul(
                wp[:, 0:256],
                wsrc[:, :, 0:P],
                wsrc[:],
                start=True,
                stop=True,
                perf_mode=perf_mode,
            )

        # ---- stage inputs in SBUF -------------------------------------
        # Both HWDGE rings dispatch inputs concurrently: sync takes zi +
        # zj k=0, scalar takes zj k=1 (its ACT_TABLE_LOAD head costs
        # ~1.3us, which the k=1 chunks hide behind). Ordered by first use.
        zi_sb = zpool.tile([P, 2, 2, NI], FP8, tag="zi", name="zi")
        zj_sb = [
            zpool.tile([P, 2, N], FP8, tag=f"zj{k}", name=f"zj{k}")
            for k in range(2)
        ]
        nc.sync.dma_start(zi_sb[:, :, :, 0:NI // 2], zi_t[0])
        nc.sync.dma_start(zj_sb[0][:, :, 0:HC], zjh_t[0][:, :, 0:HC])
        nc.scalar.dma_start(zj_sb[1][:, :, 0:HC], zjh_t[1][:, :, 0:HC])
        nc.sync.dma_start(zj_sb[0][:, :, HC:CCG], zjh_t[0][:, :, HC:CCG])
        nc.scalar.dma_start(zj_sb[1][:, :, HC:CCG], zjh_t[1][:, :, HC:CCG])
        nc.sync.dma_start(zi_sb[:, :, :, NI // 2:NI], zi_t[1])
        for g in range(1, NCCG):
            c0, c1 = g * CCG, (g + 1) * CCG
            nc.sync.dma_start(zj_sb[0][:, :, c0:c1], zjh_t[0][:, :, c0:c1])
            nc.scalar.dma_start(zj_sb[1][:, :, c0:c1], zjh_t[1][:, :, c0:c1])

        colboth = apool.tile([P, NCCG * KC], BF16, tag="colboth")
        nc.gpsimd.memset(colboth[:], 0)
        rows_sb = apool.tile([P, NS], F32, tag="rows")

        # ---- main loop ------------------------------------------------
        # colboth adds are deferred one iteration: on the in-order DVE
        # queue an add waits on its EXP sem, and issuing it before the
        # next schrd would delay gpB's PSUM release (PE stalls).
        pending = None  # (et, g) awaiting its colboth adds

        def _flush(pending):
            pet, pg = pending
            a0 = pg * KC
            nc.vector.tensor_add(
                colboth[:, a0:a0 + KCV],
                colboth[:, a0:a0 + KCV],
                pet[:, 0:KCV],
            )
            nc.gpsimd.tensor_add(
                colboth[:, a0 + KCV:a0 + KC],
                colboth[:, a0 + KCV:a0 + KC],
                pet[:, KCV:KC],
            )

        etf2 = None
        for g in range(NCCG):
            c0 = g * CCG
            for rc in range(RC):
                slot = rc * NCCG + g
                last = g == NCCG - 1 and rc == RC - 1
                gpa = psa.tile([P, HC], F32, tag="GA")
                gpb = psb.tile([P, HC], F32, tag="GB")
                # last group: fill gpB first so the fast-exp chain
                # (the long pole at drain time) starts two matmuls sooner
                ccs = (2, 3, 0, 1) if g == NCCG - 1 else (0, 1, 2, 3)
                for k in range(2):
                    lhsT = zi_sb[:, k, :, rc * P:(rc + 1) * P]
                    for cc in ccs:
                        gp = gpa if cc < 2 else gpb
                        o = (cc % 2) * MMN
                        rhs = zj_sb[k][:, :, c0 + cc * MMN:c0 + (cc + 1) * MMN]
                        nc.tensor.matmul(
                            gp[:, o:o + MMN],
                            lhsT,
                            rhs,
                            start=(k == 0),
                            stop=(k == 1),
                            perf_mode=perf_mode,
                        )

                # fast exp of gpB; two row chunks share one tile so the
                # outbound ship is a single 256KB DMA
                if rc % 2 == 0:
                    etf2 = epool.tile([P, 2, HC], U8, tag="EF")
                nc.vector.tensor_scalar(
                    etf2[:, rc % 2, :],
                    gpb[:],
                    SCHRAUDOLPH_A,
                    SCHRAUDOLPH_B,
                    mybir.AluOpType.mult,
                    mybir.AluOpType.add,
                )
                if rc % 2 == 1:
                    nc.sync.dma_start(etf_d[g, rc // 2], etf2[:])

                # table exp of gpA + fused row-partials
                et = epool.tile([P, HC], BF16, tag="E")
                nc.scalar.activation(
                    et[:],
                    gpa[:],
                    mybir.ActivationFunctionType.Exp,
                    bias=0.0,
                    scale=ES,
                    accum_out=rows_sb[:, slot:slot + 1],
                )
                if pending is not None:
                    pg = pending[1]
                    _flush(pending)
                    if pg != g:  # previous group's columns complete
                        nc.sync.dma_start(
                            cols_d[pg], colboth[:, pg * KC:(pg + 1) * KC]
                        )
                if last:
                    # final slot: ship et raw (host adds it into the g=3
                    # column sums) instead of chaining two accumulates +
                    # a dependent ship onto the drain tail
                    pending = None
                    nc.scalar.dma_start(etl_d[:, :], et[:])
                else:
                    pending = (et, g)

        if pending is not None:
            _flush(pending)
        nc.sync.dma_start(
            cols_d[NCCG - 1], colboth[:, (NCCG - 1) * KC:NCCG * KC]
        )
        nc.scalar.dma_start(rows_d[:, :], rows_sb[:])


def _get_nc():
    if "nc" not in _compiled:
        _compiled["nc"] = _build()
    return _compiled["nc"]


def _pack_fp8_zi(zt):
    """[D, NI] fp32 -> [2, P, 2, 2, NI//2] fp8 with d = k*256 + s*128 + p
    and col = half*(NI//2) + c; each half chunk is fully contiguous."""
    ni = zt.shape[1]
    return np.ascontiguousarray(
        (zt * FP8_SCALE).reshape(2, 2, P, 2, ni // 2).transpose(3, 2, 0, 1, 4)
    ).astype(NP_FP8)


def _pack_fp8_zj(zt):
    """[D, N] fp32 -> [2, P, 2, N] fp8 with d = k*256 + s*128 + p; every
    dispatch slice [k, :, :, a:b] has (b-a)-byte descriptor runs."""
    return np.ascontiguousarray(
        (zt * FP8_SCALE).reshape(2, 2, P, N).transpose(0, 2, 1, 3)
    ).astype(NP_FP8)


def _prep_inputs(z_i, z_j):
    """Host-side sharding: normalize (fp32, as the reference), transpose to
    [D, N] (the layout the PE contracts over), quantize, slice per core."""
    zi = np.asarray(z_i, dtype=np.float32)
    zj = np.asarray(z_j, dtype=np.float32)
    ni = np.maximum(np.sqrt((zi * zi).sum(-1, keepdims=True)), EPS)
    nj = np.maximum(np.sqrt((zj * zj).sum(-1, keepdims=True)), EPS)
    zin = zi / ni
    zjn = zj / nj
    pos = (zin * zjn).sum(-1, dtype=np.float64) / TAU  # diagonal of sim, [N]

    zin_t = zin.T  # [D, N]
    zjn_t = zjn.T

    zjh = _pack_fp8_zj(zjn_t)
    in_maps = []
    for c in range(NCORES):
        in_maps.append(
            {
                "zi_t": _pack_fp8_zi(zin_t[:, c * NI:(c + 1) * NI]),
                "zjh_t": zjh,
            }
        )
    return in_maps, pos


def _reduce_core(out):
    """Device outputs of one core -> (rowsum[NI], colsum[N]) in fp64."""
    rows = out["rowsums"].astype(np.float64)          # [128, NS], cols [0:HC]
    etf = (
        out["etf"].view(ml_dtypes.float8_e4m3fn).astype(np.float64)
        .transpose(0, 1, 3, 2, 4)                     # [NCCG, RC//2, 2, P, HC]
        .reshape(NCCG, RC, P, HC)                     # rc = 2*rp + j
    )
    # rowsum: scalar-slice accum per (rc, g) + host-reduced fast slice
    per_rc = rows.reshape(P, RC, NCCG).sum(-1)        # [p, rc]
    per_rc = per_rc + etf.sum(-1).sum(0).T            # fast slice, [p, rc]
    rowsum = per_rc.T.reshape(-1)                     # global row = rc*128+p
    # colsum: device colboth + host-summed raw last tile + etf fast slice
    colacc = out["colacc"].astype(np.float64)         # [NCCG, 128, KC]
    colsum = np.empty(N, dtype=np.float64)
    cview = colsum.reshape(NCCG, CCG)
    cview[:, 0:KC] = colacc.sum(1)
    cview[NCCG - 1, 0:KC] += out["etl"].astype(np.float64).sum(0)
    cview[:, KC:] = etf.sum((1, 2))
    return rowsum, colsum


def kernel(z_i, z_j):
    global LAST_RESULTS
    in_maps, pos = _prep_inputs(z_i, z_j)
    nc = _get_nc()

    res = bass_utils.run_bass_kernel_spmd(nc, in_maps, core_ids=list(range(NCORES)))
    LAST_RESULTS = res

    rowsum = np.zeros(N, dtype=np.float64)
    colsum = np.zeros(N, dtype=np.float64)
    for c in range(NCORES):
        r, cs = _reduce_core(res.results[c])
        rowsum[c * NI:(c + 1) * NI] = r
        colsum += cs

    # host-side "all-reduce" epilogue: drop the diagonal, logs, means
    exp_pos = np.exp(pos)
    lse_row = np.log(rowsum - exp_pos)
    lse_col = np.log(colsum - exp_pos)
    loss_e2t = np.mean(lse_row - pos)
    loss_t2e = np.mean(lse_col - pos)
    loss = 0.5 * (loss_e2t + loss_t2e)
    return np.stack([loss, loss_e2t, loss_t2e]).astype(np.float32)
